# revision 7
# baseline (speedup 1.0000x reference)
"""Trainium2 Bass kernel for BiLSTM pairwise model (nn_BiLSTM_45612552684167).

Strategy:
  - 2-layer bidirectional LSTM + MLP replicated on all 8 cores (the LSTM
    recurrence is latency-bound; replication avoids collectives entirely).
  - Pairwise [Nr, Nl] grid sharded along Nr via partition_id: each core
    computes its 48-row block of relu(u_r[i]+u_l[j]+b3) and the final
    projection, exploiting RRI=2: log_softmax reduces to softplus of
    Delta = logit1 - logit0 (single matvec against Wout[1]-Wout[0]).
  - bf16 on PE-facing tensors, f32 PSUM accumulation and elementwise.
Layouts keep every activation transposed ([feature-chunk(128), time/pair])
so ACT per-partition bias == feature bias and matmuls need no transposes.
"""

import sys
from contextlib import ExitStack

sys.path.insert(0, "/opt/trn_rl_repo")

import numpy as np
import ml_dtypes

import concourse.bass as bass
import concourse.mybir as mybir
import concourse.tile as tile
from concourse import bacc
from concourse.bass import ds
from concourse.bass_utils import run_bass_kernel_spmd

BFNP = ml_dtypes.bfloat16
F32 = mybir.dt.float32
BF16 = mybir.dt.bfloat16
AF = mybir.ActivationFunctionType
ALU = mybir.AluOpType

DIN = 22
H = 256
G = 1024  # 4*H
H1, H2, H3 = 1024, 512, 1024
NCORES = 8

_cache = {}


def _gate_perm():
    # torch gate order i,f,g,o -> device order i,f,o,g (sigmoid block contiguous)
    idx = np.arange(G).reshape(4, H)
    return np.concatenate([idx[0], idx[1], idx[3], idx[2]])


def _build(T):
    RB = T // NCORES
    nc = bacc.Bacc("TRN2", target_bir_lowering=False, debug=False, num_devices=NCORES)

    def inp(name, shape, dt):
        return nc.declare_dram_parameter(name, list(shape), dt, isOutput=False)

    XT = inp("XT", [DIN, 2 * T], BF16)
    WIH0T = inp("WIH0T", [2, DIN, G], BF16)
    WHH0T = inp("WHH0T", [2, 128, 2048], BF16)
    WIH1T = inp("WIH1T", [2, 128, 4096], BF16)
    WHH1T = inp("WHH1T", [2, 128, 2048], BF16)
    B0 = inp("B0", [2, 128, 8], F32)
    B1R = inp("B1R", [2, 128, 8], F32)
    W1T = inp("W1T", [128, 4096], BF16)  # tiles (k4, m8)
    B1M = inp("B1M", [128, 8], F32)
    W2T = inp("W2T", [128, 4096], BF16)  # tiles (k8, m4)
    B2M = inp("B2M", [128, 4], F32)
    W3T = inp("W3T", [128, 4096], BF16)  # tiles (k4, m8), pre-scaled 0.5
    B3 = inp("B3", [128, 8], F32)
    WDP = inp("WDP", [128, 16], BF16)  # per m-chunk: [wd, -wd]
    BDP = inp("BDP", [1, 2], BF16)  # [bd, -bd]
    IDN = inp("IDN", [128, 128], BF16)
    OUT = nc.declare_dram_parameter("OUT", [2, RB * T], F32, isOutput=True)

    with tile.TileContext(nc) as tc, ExitStack() as _es:
        sp = _es.enter_context(tc.tile_pool(name="static", bufs=1))
        wk = _es.enter_context(tc.tile_pool(name="work", bufs=3))
        pg = _es.enter_context(tc.tile_pool(name="psg", bufs=2, space="PSUM"))
        pb = _es.enter_context(tc.tile_pool(name="psb", bufs=2, space="PSUM"))
        pd = _es.enter_context(tc.tile_pool(name="psd", bufs=2, space="PSUM"))

        # ---- load all inputs to SBUF ----
        def load(name, dram_ap, shape, dt):
            t_ = sp.tile(shape, dt, tag=name)
            nc.sync.dma_start(t_[:], dram_ap)
            return t_

        xt = load("xt", XT[:, :], [DIN, 2 * T], BF16)
        wih0 = [load(f"wih0_{d}", WIH0T[d, :, :], [DIN, G], BF16) for d in (0, 1)]
        whh0 = [load(f"whh0_{d}", WHH0T[d, :, :], [128, 2048], BF16) for d in (0, 1)]
        wih1 = [load(f"wih1_{d}", WIH1T[d, :, :], [128, 4096], BF16) for d in (0, 1)]
        whh1 = [load(f"whh1_{d}", WHH1T[d, :, :], [128, 2048], BF16) for d in (0, 1)]
        b0 = [load(f"b0_{d}", B0[d, :, :], [128, 8], F32) for d in (0, 1)]
        b1r = [load(f"b1r_{d}", B1R[d, :, :], [128, 8], F32) for d in (0, 1)]
        w1t = load("w1t", W1T[:, :], [128, 4096], BF16)
        b1m = load("b1m", B1M[:, :], [128, 8], F32)
        w2t = load("w2t", W2T[:, :], [128, 4096], BF16)
        b2m = load("b2m", B2M[:, :], [128, 4], F32)
        w3t = load("w3t", W3T[:, :], [128, 4096], BF16)
        b3 = load("b3", B3[:, :], [128, 8], F32)
        wdp = load("wdp", WDP[:, :], [128, 16], BF16)
        bdp = load("bdp", BDP[:, :], [1, 2], BF16)
        idn = load("idn", IDN[:, :], [128, 128], BF16)

        ones = sp.tile([1, T], BF16, name="ones", tag="ones")
        nc.gpsimd.memset(ones[:], 1.0)

        pre_a = sp.tile([128, 32 * T], BF16, name="pre_a", tag="pre_a")
        pre_b = pre_a
        hist0 = [sp.tile([128, 4 * T], BF16, name=f"hist0_{d}", tag=f"hist0_{d}") for d in (0, 1)]
        hist1 = [sp.tile([128, 4 * T], BF16, name=f"hist1_{d}", tag=f"hist1_{d}") for d in (0, 1)]
        cst = [sp.tile([128, 4], F32, name=f"c_{d}", tag=f"c_{d}") for d in (0, 1)]

        def build_pre_a():
            # pre_a[:, t*32 + d*16 + m*2 + s] = (Wih0[d] @ x_s[t])[mchunk] + b0[d][mchunk]
            pre_r = pre_a.rearrange("p (t q) -> p t q", q=32)
            for d in (0, 1):
                for s in (0, 1):
                    for m in range(8):
                        ps = pb.tile([128, T], F32, name="big", tag="big")
                        nc.tensor.matmul(
                            ps[:],
                            wih0[d][:, m * 128 : (m + 1) * 128],
                            xt[:, s * T : (s + 1) * T],
                            start=True,
                            stop=True,
                        )
                        dst = pre_r[:, :, d * 16 + m * 2 + s]
                        if (d + s + m) % 2 == 0:
                            nc.scalar.activation(
                                dst, ps[:], AF.Identity, bias=b0[d][:, m : m + 1]
                            )
                        else:
                            nc.vector.tensor_scalar(
                                dst, ps[:], b0[d][:, m : m + 1], None, ALU.add
                            )

        def build_pre_b():
            # x1 = [h_fwd, h_bwd] per seq; pre_b from Wih1 + b1r
            pre_r = pre_b.rearrange("p (t q) -> p t q", q=32)
            h0r = [hist0[dd].rearrange("p (t q) -> p t q", q=4) for dd in (0, 1)]
            for d in (0, 1):
                for s in (0, 1):
                    for m in range(8):
                        ps = pb.tile([128, T], F32, name="big", tag="big")
                        for k in range(4):
                            rhs = h0r[k // 2][:, :, (k % 2) * 2 + s]
                            nc.tensor.matmul(
                                ps[:],
                                wih1[d][:, (k * 8 + m) * 128 : (k * 8 + m + 1) * 128],
                                rhs,
                                start=(k == 0),
                                stop=(k == 3),
                                skip_group_check=True,
                            )
                        dst = pre_r[:, :, d * 16 + m * 2 + s]
                        if (d + s + m) % 2 == 0:
                            nc.scalar.activation(
                                dst, ps[:], AF.Identity, bias=b1r[d][:, m : m + 1]
                            )
                        else:
                            nc.vector.tensor_scalar(
                                dst, ps[:], b1r[d][:, m : m + 1], None, ALU.add
                            )

        def lstm_phase(pre, whh, hist):
            for d in (0, 1):
                nc.gpsimd.memset(cst[d][:], 0.0)
            for t in range(T):
                for d in (0, 1):
                    tau = t if d == 0 else T - 1 - t
                    ps = pg.tile([128, 16], F32, name=f"g{d}", tag=f"g{d}")
                    off = tau * 32 + d * 16
                    nc.tensor.matmul(
                        ps[:],
                        idn[:],
                        pre[:, off : off + 16],
                        start=True,
                        stop=(t == 0),
                        skip_group_check=True,
                    )
                    if t > 0:
                        ptau = tau - 1 if d == 0 else tau + 1
                        for k in (0, 1):
                            rhs = hist[d][:, ptau * 4 + k * 2 : ptau * 4 + k * 2 + 2]
                            for m in range(8):
                                nc.tensor.matmul(
                                    ps[:, m * 2 : m * 2 + 2],
                                    whh[d][:, (k * 8 + m) * 128 : (k * 8 + m + 1) * 128],
                                    rhs,
                                    start=False,
                                    stop=(k == 1 and m == 7),
                                    skip_group_check=True,
                                )
                    gsb = wk.tile([128, 16], F32, name=f"gs{d}", tag=f"gs{d}")
                    nc.scalar.activation(gsb[:, 0:12], ps[:, 0:12], AF.Sigmoid)
                    nc.scalar.activation(gsb[:, 12:16], ps[:, 12:16], AF.Tanh)
                    tmp = wk.tile([128, 4], F32, name=f"tmp{d}", tag=f"tmp{d}")
                    nc.vector.tensor_tensor(
                        tmp[:], gsb[:, 0:4], gsb[:, 12:16], ALU.mult
                    )
                    nc.vector.tensor_tensor(
                        cst[d][:], gsb[:, 4:8], cst[d][:], ALU.mult
                    )
                    nc.vector.tensor_tensor(cst[d][:], cst[d][:], tmp[:], ALU.add)
                    tch = wk.tile([128, 4], F32, name=f"tc{d}", tag=f"tc{d}")
                    nc.scalar.activation(tch[:], cst[d][:], AF.Tanh)
                    nc.vector.tensor_tensor(
                        hist[d][:, tau * 4 : tau * 4 + 4],
                        gsb[:, 8:12],
                        tch[:],
                        ALU.mult,
                    )

        build_pre_a()
        lstm_phase(pre_a, whh0, hist0)
        build_pre_b()
        lstm_phase(pre_b, whh1, hist1)

        # ---- MLP (transposed activations) ----
        h1t = [sp.tile([128, 8 * T], BF16, name=f"h1t_{s}", tag=f"h1t_{s}") for s in (0, 1)]
        h2t = [sp.tile([128, 4 * T], BF16, name=f"h2t_{s}", tag=f"h2t_{s}") for s in (0, 1)]
        urt = sp.tile([128, 8 * T], BF16, name="urt", tag="urt")
        ult = sp.tile([128, 8 * T], BF16, name="ult", tag="ult")
        h1r = [hist1[dd].rearrange("p (t q) -> p t q", q=4) for dd in (0, 1)]

        for s in (0, 1):
            for m in range(8):
                ps = pb.tile([128, T], F32, name="big", tag="big")
                for k in range(4):
                    rhs = h1r[k // 2][:, :, (k % 2) * 2 + s]
                    nc.tensor.matmul(
                        ps[:],
                        w1t[:, (k * 8 + m) * 128 : (k * 8 + m + 1) * 128],
                        rhs,
                        start=(k == 0),
                        stop=(k == 3),
                        skip_group_check=True,
                    )
                nc.scalar.activation(
                    h1t[s][:, m * T : (m + 1) * T],
                    ps[:],
                    AF.Relu,
                    bias=b1m[:, m : m + 1],
                )
            for m in range(4):
                ps = pb.tile([128, T], F32, name="big", tag="big")
                for k in range(8):
                    nc.tensor.matmul(
                        ps[:],
                        w2t[:, (k * 4 + m) * 128 : (k * 4 + m + 1) * 128],
                        h1t[s][:, k * T : (k + 1) * T],
                        start=(k == 0),
                        stop=(k == 7),
                        skip_group_check=True,
                    )
                nc.scalar.activation(
                    h2t[s][:, m * T : (m + 1) * T],
                    ps[:],
                    AF.Relu,
                    bias=b2m[:, m : m + 1],
                )
            dst_u = urt if s == 0 else ult
            for m in range(8):
                ps = pb.tile([128, T], F32, name="big", tag="big")
                for k in range(4):
                    nc.tensor.matmul(
                        ps[:],
                        w3t[:, (k * 8 + m) * 128 : (k * 8 + m + 1) * 128],
                        h2t[s][:, k * T : (k + 1) * T],
                        start=(k == 0),
                        stop=(k == 3),
                        skip_group_check=True,
                    )
                if s == 1:
                    nc.scalar.activation(
                        dst_u[:, m * T : (m + 1) * T],
                        ps[:],
                        AF.Identity,
                        bias=b3[:, m : m + 1],
                    )
                else:
                    nc.scalar.activation(
                        dst_u[:, m * T : (m + 1) * T], ps[:], AF.Identity, bias=0.0
                    )

        # ---- pairwise block (rows pid*RB .. pid*RB+RB-1) ----
        urm = sp.tile([128, 8 * RB], F32, name="urm", tag="urm")
        pid = nc.vector.partition_id()
        urt_r = urt.rearrange("p (m t) -> p m t", m=8)
        urm_r = urm.rearrange("p (m t) -> p m t", m=8)
        nc.vector.tensor_copy(urm_r[:, :, :], urt_r[:, :, ds(pid * RB, RB)])

        for i in range(RB):
            ps = pd.tile([2, T], F32, name="pdl", tag="pdl")
            nc.tensor.matmul(
                ps[:], bdp[:], ones[:], start=True, stop=False, skip_group_check=True
            )
            for m in range(8):
                rt = wk.tile([128, T], BF16, name="rt", tag="rt")
                src = ult[:, m * T : (m + 1) * T]
                bcol = urm[:, m * RB + i : m * RB + i + 1]
                if m < 6:
                    nc.vector.tensor_scalar(
                        rt[:], src, bcol, 0.0, ALU.add, ALU.max
                    )
                elif m == 6:
                    nc.gpsimd.tensor_scalar(
                        rt[:], src, bcol, 0.0, ALU.add, ALU.max
                    )
                else:
                    nc.scalar.activation(rt[:], src, AF.Relu, bias=bcol)
                nc.tensor.matmul(
                    ps[:],
                    wdp[:, m * 2 : (m + 1) * 2],
                    rt[:],
                    start=False,
                    stop=(m == 7),
                    skip_group_check=True,
                )
            # rows of ps: (Delta, -Delta); out1 = Delta - softplus(Delta),
            # out0 = -Delta - softplus(-Delta)  -> rows (out1, out0)
            ex = wk.tile([2, T], F32, name="ex", tag="ex")
            nc.scalar.activation(ex[:], ps[:], AF.Exp)
            ll = wk.tile([2, T], F32, name="ll", tag="ll")
            nc.scalar.activation(ll[:], ex[:], AF.Ln, bias=1.0)
            xo = wk.tile([2, T], F32, name="xo", tag="xo")
            nc.vector.tensor_tensor(xo[:], ps[:], ll[:], ALU.subtract)
            nc.sync.dma_start(OUT[:, i * T : (i + 1) * T], xo[:])

    nc.compile()
    return nc


def kernel(**inputs):
    return _kernel_impl(T=384, **inputs)


def _kernel_impl(T, v_r, v_l, Wih0, Whh0, bih0, bhh0, Wih1, Whh1, bih1, bhh1,
                 W1, b1, W2, b2, W3, b3, Wout, bout):
    RB = T // NCORES
    perm = _gate_perm()

    def bf(x):
        return np.ascontiguousarray(np.asarray(x, np.float32)).astype(BFNP)

    def f32(x):
        return np.ascontiguousarray(np.asarray(x, np.float32))

    def tiles_km(wt, nk, nm):
        outp = np.zeros((128, nk * nm * 128), np.float32)
        for k in range(nk):
            for m in range(nm):
                blk = wt[k * 128 : (k + 1) * 128, m * 128 : (m + 1) * 128]
                outp[: blk.shape[0], (k * nm + m) * 128 : (k * nm + m) * 128 + blk.shape[1]] = blk
        return outp

    v_r, v_l = np.asarray(v_r, np.float32), np.asarray(v_l, np.float32)
    Wih0, Whh0 = np.asarray(Wih0, np.float32), np.asarray(Whh0, np.float32)
    Wih1, Whh1 = np.asarray(Wih1, np.float32), np.asarray(Whh1, np.float32)
    b0 = np.asarray(bih0, np.float32) + np.asarray(bhh0, np.float32)
    b1r = np.asarray(bih1, np.float32) + np.asarray(bhh1, np.float32)
    W1, b1 = np.asarray(W1, np.float32), np.asarray(b1, np.float32)
    W2, b2 = np.asarray(W2, np.float32), np.asarray(b2, np.float32)
    W3, b3 = np.asarray(W3, np.float32), np.asarray(b3, np.float32)
    Wout, bout = np.asarray(Wout, np.float32), np.asarray(bout, np.float32)

    xt = np.concatenate([v_r.T, v_l.T], axis=1)
    wih0t = np.stack([Wih0[d][perm].T for d in (0, 1)])
    b0p = np.stack([b0[d][perm].reshape(8, 128).T for d in (0, 1)])
    whh0t = np.stack([tiles_km(Whh0[d][perm].T, 2, 8) for d in (0, 1)])
    wih1t = np.stack([tiles_km(Wih1[d][perm].T, 4, 8) for d in (0, 1)])
    b1rp = np.stack([b1r[d][perm].reshape(8, 128).T for d in (0, 1)])
    whh1t = np.stack([tiles_km(Whh1[d][perm].T, 2, 8) for d in (0, 1)])
    w1tt = tiles_km(W1.T, 4, 8)
    b1mp = b1.reshape(8, 128).T
    w2tt = tiles_km(W2.T, 8, 4)
    b2mp = b2.reshape(4, 128).T
    w3s = 0.5 * (W3[:, :H2] + W3[:, H2:]).T
    w3tt = tiles_km(w3s, 4, 8)
    b3p = b3.reshape(8, 128).T
    wd = Wout[1] - Wout[0]
    wdp = np.zeros((128, 16), np.float32)
    for m in range(8):
        wdp[:, m * 2] = wd[m * 128 : (m + 1) * 128]
        wdp[:, m * 2 + 1] = -wd[m * 128 : (m + 1) * 128]
    bd = float(bout[1] - bout[0])
    bdp = np.array([[bd, -bd]], np.float32)

    in_map = {
        "XT": bf(xt),
        "WIH0T": bf(wih0t),
        "WHH0T": bf(whh0t),
        "WIH1T": bf(wih1t),
        "WHH1T": bf(whh1t),
        "B0": f32(b0p),
        "B1R": f32(b1rp),
        "W1T": bf(w1tt),
        "B1M": f32(b1mp),
        "W2T": bf(w2tt),
        "B2M": f32(b2mp),
        "W3T": bf(w3tt),
        "B3": f32(b3p),
        "WDP": bf(wdp),
        "BDP": bf(bdp),
        "IDN": bf(np.eye(128, dtype=np.float32)),
    }

    if T not in _cache:
        _cache[T] = _build(T)
    nc = _cache[T]

    core_ids = list(range(NCORES))
    in_maps = [in_map for _ in core_ids]
    res = run_bass_kernel_spmd(nc, in_maps, core_ids)

    out = np.empty((T, T, 2), np.float32)
    for c in core_ids:
        o = res.results[c]["OUT"].reshape(2, RB, T)
        out[c * RB : (c + 1) * RB, :, 0] = o[1]
        out[c * RB : (c + 1) * RB, :, 1] = o[0]
    return out.reshape(T * T, 2)


# revision 9
# speedup vs baseline: 1.0170x; 1.0170x over previous
"""Trainium2 Bass kernel for BiLSTM pairwise model (nn_BiLSTM_45612552684167).

Strategy:
  - 2-layer bidirectional LSTM + MLP replicated on all 8 cores (the LSTM
    recurrence is latency-bound; replication avoids collectives entirely).
  - Pairwise [Nr, Nl] grid sharded along Nr via partition_id: each core
    computes its 48-row block of relu(u_r[i]+u_l[j]+b3) and the final
    projection, exploiting RRI=2: log_softmax reduces to softplus of
    Delta = logit1 - logit0 (single matvec against Wout[1]-Wout[0]).
  - bf16 on PE-facing tensors, f32 PSUM accumulation and elementwise.
Layouts keep every activation transposed ([feature-chunk(128), time/pair])
so ACT per-partition bias == feature bias and matmuls need no transposes.
"""

import sys
from contextlib import ExitStack

sys.path.insert(0, "/opt/trn_rl_repo")

import numpy as np
import ml_dtypes

import concourse.bass as bass
import concourse.mybir as mybir
import concourse.tile as tile
from concourse import bacc
from concourse.bass import ds
from concourse.bass_utils import run_bass_kernel_spmd

BFNP = ml_dtypes.bfloat16
F32 = mybir.dt.float32
BF16 = mybir.dt.bfloat16
AF = mybir.ActivationFunctionType
ALU = mybir.AluOpType

DIN = 22
H = 256
G = 1024  # 4*H
H1, H2, H3 = 1024, 512, 1024
NCORES = 8

_cache = {}


def _gate_perm():
    # torch gate order i,f,g,o -> device order i,f,o,g (sigmoid block contiguous)
    idx = np.arange(G).reshape(4, H)
    return np.concatenate([idx[0], idx[1], idx[3], idx[2]])


def _build(T):
    RB = T // NCORES
    nc = bacc.Bacc("TRN2", target_bir_lowering=False, debug=False, num_devices=NCORES)

    def inp(name, shape, dt):
        return nc.declare_dram_parameter(name, list(shape), dt, isOutput=False)

    XT = inp("XT", [DIN, 2 * T], BF16)
    WIH0T = inp("WIH0T", [2, DIN, G], BF16)
    WHH0T = inp("WHH0T", [2, 128, 2048], BF16)
    WIH1T = inp("WIH1T", [2, 128, 4096], BF16)
    WHH1T = inp("WHH1T", [2, 128, 2048], BF16)
    B0 = inp("B0", [2, 128, 8], F32)
    B1R = inp("B1R", [2, 128, 8], F32)
    W1T = inp("W1T", [128, 4096], BF16)  # tiles (k4, m8)
    B1M = inp("B1M", [128, 8], F32)
    W2T = inp("W2T", [128, 4096], BF16)  # tiles (k8, m4)
    B2M = inp("B2M", [128, 4], F32)
    W3T = inp("W3T", [128, 4096], BF16)  # tiles (k4, m8), pre-scaled 0.5
    B3 = inp("B3", [128, 8], F32)
    WDP = inp("WDP", [128, 16], BF16)  # per m-chunk: [wd, -wd]
    BDP = inp("BDP", [1, 2], BF16)  # [bd, -bd]
    IDN = inp("IDN", [128, 128], BF16)
    OUT = nc.declare_dram_parameter("OUT", [2, RB * T], F32, isOutput=True)

    with tile.TileContext(nc) as tc, ExitStack() as _es:
        sp = _es.enter_context(tc.tile_pool(name="static", bufs=1))
        wk = _es.enter_context(tc.tile_pool(name="work", bufs=4))
        pg = _es.enter_context(tc.tile_pool(name="psg", bufs=2, space="PSUM"))
        pb = _es.enter_context(tc.tile_pool(name="psb", bufs=2, space="PSUM"))
        pd = _es.enter_context(tc.tile_pool(name="psd", bufs=2, space="PSUM"))

        # ---- load all inputs to SBUF ----
        def load(name, dram_ap, shape, dt):
            t_ = sp.tile(shape, dt, tag=name)
            nc.sync.dma_start(t_[:], dram_ap)
            return t_

        xt = load("xt", XT[:, :], [DIN, 2 * T], BF16)
        wih0 = [load(f"wih0_{d}", WIH0T[d, :, :], [DIN, G], BF16) for d in (0, 1)]
        whh0 = [load(f"whh0_{d}", WHH0T[d, :, :], [128, 2048], BF16) for d in (0, 1)]
        wih1 = [load(f"wih1_{d}", WIH1T[d, :, :], [128, 4096], BF16) for d in (0, 1)]
        whh1 = [load(f"whh1_{d}", WHH1T[d, :, :], [128, 2048], BF16) for d in (0, 1)]
        b0 = [load(f"b0_{d}", B0[d, :, :], [128, 8], F32) for d in (0, 1)]
        b1r = [load(f"b1r_{d}", B1R[d, :, :], [128, 8], F32) for d in (0, 1)]
        w1t = load("w1t", W1T[:, :], [128, 4096], BF16)
        b1m = load("b1m", B1M[:, :], [128, 8], F32)
        w2t = load("w2t", W2T[:, :], [128, 4096], BF16)
        b2m = load("b2m", B2M[:, :], [128, 4], F32)
        w3t = load("w3t", W3T[:, :], [128, 4096], BF16)
        b3 = load("b3", B3[:, :], [128, 8], F32)
        wdp = load("wdp", WDP[:, :], [128, 16], BF16)
        bdp = load("bdp", BDP[:, :], [1, 2], BF16)
        idn = load("idn", IDN[:, :], [128, 128], BF16)

        ones = sp.tile([1, T], BF16, name="ones", tag="ones")
        nc.gpsimd.memset(ones[:], 1.0)

        pre_a = sp.tile([128, 32 * T], BF16, name="pre_a", tag="pre_a")
        pre_b = pre_a
        hist0 = [sp.tile([128, 4 * T], BF16, name=f"hist0_{d}", tag=f"hist0_{d}") for d in (0, 1)]
        hist1 = [sp.tile([128, 4 * T], BF16, name=f"hist1_{d}", tag=f"hist1_{d}") for d in (0, 1)]
        cst = [sp.tile([128, 4], F32, name=f"c_{d}", tag=f"c_{d}") for d in (0, 1)]

        def build_pre_a():
            # pre_a[:, t*32 + d*16 + m*2 + s] = (Wih0[d] @ x_s[t])[mchunk] + b0[d][mchunk]
            pre_r = pre_a.rearrange("p (t q) -> p t q", q=32)
            for d in (0, 1):
                for s in (0, 1):
                    for m in range(8):
                        ps = pb.tile([128, T], F32, name="big", tag="big")
                        nc.tensor.matmul(
                            ps[:],
                            wih0[d][:, m * 128 : (m + 1) * 128],
                            xt[:, s * T : (s + 1) * T],
                            start=True,
                            stop=True,
                        )
                        dst = pre_r[:, :, d * 16 + m * 2 + s]
                        if (d + s + m) % 2 == 0:
                            nc.scalar.activation(
                                dst, ps[:], AF.Identity, bias=b0[d][:, m : m + 1]
                            )
                        else:
                            nc.vector.tensor_scalar(
                                dst, ps[:], b0[d][:, m : m + 1], None, ALU.add
                            )

        def build_pre_b():
            # x1 = [h_fwd, h_bwd] per seq; pre_b from Wih1 + b1r
            pre_r = pre_b.rearrange("p (t q) -> p t q", q=32)
            h0r = [hist0[dd].rearrange("p (t q) -> p t q", q=4) for dd in (0, 1)]
            for d in (0, 1):
                for s in (0, 1):
                    for m in range(8):
                        ps = pb.tile([128, T], F32, name="big", tag="big")
                        for k in range(4):
                            rhs = h0r[k // 2][:, :, (k % 2) * 2 + s]
                            nc.tensor.matmul(
                                ps[:],
                                wih1[d][:, (k * 8 + m) * 128 : (k * 8 + m + 1) * 128],
                                rhs,
                                start=(k == 0),
                                stop=(k == 3),
                                skip_group_check=True,
                            )
                        dst = pre_r[:, :, d * 16 + m * 2 + s]
                        if (d + s + m) % 2 == 0:
                            nc.scalar.activation(
                                dst, ps[:], AF.Identity, bias=b1r[d][:, m : m + 1]
                            )
                        else:
                            nc.vector.tensor_scalar(
                                dst, ps[:], b1r[d][:, m : m + 1], None, ALU.add
                            )

        def lstm_phase(pre, whh, hist):
            for d in (0, 1):
                nc.gpsimd.memset(cst[d][:], 0.0)
            for t in range(T):
                for d in (0, 1):
                    tau = t if d == 0 else T - 1 - t
                    ps = pg.tile([128, 16], F32, name=f"g{d}", tag=f"g{d}")
                    off = tau * 32 + d * 16
                    nc.tensor.matmul(
                        ps[:],
                        idn[:],
                        pre[:, off : off + 16],
                        start=True,
                        stop=(t == 0),
                        skip_group_check=True,
                    )
                    if t > 0:
                        ptau = tau - 1 if d == 0 else tau + 1
                        for k in (0, 1):
                            rhs = hist[d][:, ptau * 4 + k * 2 : ptau * 4 + k * 2 + 2]
                            for m in range(8):
                                nc.tensor.matmul(
                                    ps[:, m * 2 : m * 2 + 2],
                                    whh[d][:, (k * 8 + m) * 128 : (k * 8 + m + 1) * 128],
                                    rhs,
                                    start=False,
                                    stop=(k == 1 and m == 7),
                                    skip_group_check=True,
                                )
                    gsb = wk.tile([128, 16], F32, name=f"gs{d}", tag=f"gs{d}")
                    nc.scalar.activation(gsb[:, 0:12], ps[:, 0:12], AF.Sigmoid)
                    nc.scalar.activation(gsb[:, 12:16], ps[:, 12:16], AF.Tanh)
                    tmp = wk.tile([128, 4], F32, name=f"tmp{d}", tag=f"tmp{d}")
                    nc.vector.tensor_tensor(
                        tmp[:], gsb[:, 0:4], gsb[:, 12:16], ALU.mult
                    )
                    nc.vector.tensor_tensor(
                        cst[d][:], gsb[:, 4:8], cst[d][:], ALU.mult
                    )
                    nc.vector.tensor_tensor(cst[d][:], cst[d][:], tmp[:], ALU.add)
                    tch = wk.tile([128, 4], F32, name=f"tc{d}", tag=f"tc{d}")
                    nc.scalar.activation(tch[:], cst[d][:], AF.Tanh)
                    nc.vector.tensor_tensor(
                        hist[d][:, tau * 4 : tau * 4 + 4],
                        gsb[:, 8:12],
                        tch[:],
                        ALU.mult,
                    )

        build_pre_a()
        lstm_phase(pre_a, whh0, hist0)
        build_pre_b()
        lstm_phase(pre_b, whh1, hist1)

        # ---- MLP (transposed activations) ----
        h1t = [sp.tile([128, 8 * T], BF16, name=f"h1t_{s}", tag=f"h1t_{s}") for s in (0, 1)]
        h2t = [sp.tile([128, 4 * T], BF16, name=f"h2t_{s}", tag=f"h2t_{s}") for s in (0, 1)]
        urt = sp.tile([128, 8 * T], BF16, name="urt", tag="urt")
        ult = sp.tile([128, 8 * T], BF16, name="ult", tag="ult")
        h1r = [hist1[dd].rearrange("p (t q) -> p t q", q=4) for dd in (0, 1)]

        for s in (0, 1):
            for m in range(8):
                ps = pb.tile([128, T], F32, name="big", tag="big")
                for k in range(4):
                    rhs = h1r[k // 2][:, :, (k % 2) * 2 + s]
                    nc.tensor.matmul(
                        ps[:],
                        w1t[:, (k * 8 + m) * 128 : (k * 8 + m + 1) * 128],
                        rhs,
                        start=(k == 0),
                        stop=(k == 3),
                        skip_group_check=True,
                    )
                nc.scalar.activation(
                    h1t[s][:, m * T : (m + 1) * T],
                    ps[:],
                    AF.Relu,
                    bias=b1m[:, m : m + 1],
                )
            for m in range(4):
                ps = pb.tile([128, T], F32, name="big", tag="big")
                for k in range(8):
                    nc.tensor.matmul(
                        ps[:],
                        w2t[:, (k * 4 + m) * 128 : (k * 4 + m + 1) * 128],
                        h1t[s][:, k * T : (k + 1) * T],
                        start=(k == 0),
                        stop=(k == 7),
                        skip_group_check=True,
                    )
                nc.scalar.activation(
                    h2t[s][:, m * T : (m + 1) * T],
                    ps[:],
                    AF.Relu,
                    bias=b2m[:, m : m + 1],
                )
            dst_u = urt if s == 0 else ult
            for m in range(8):
                ps = pb.tile([128, T], F32, name="big", tag="big")
                for k in range(4):
                    nc.tensor.matmul(
                        ps[:],
                        w3t[:, (k * 8 + m) * 128 : (k * 8 + m + 1) * 128],
                        h2t[s][:, k * T : (k + 1) * T],
                        start=(k == 0),
                        stop=(k == 3),
                        skip_group_check=True,
                    )
                if s == 1:
                    nc.scalar.activation(
                        dst_u[:, m * T : (m + 1) * T],
                        ps[:],
                        AF.Identity,
                        bias=b3[:, m : m + 1],
                    )
                else:
                    nc.scalar.activation(
                        dst_u[:, m * T : (m + 1) * T], ps[:], AF.Identity, bias=0.0
                    )

        # ---- pairwise block (rows pid*RB .. pid*RB+RB-1) ----
        urm = sp.tile([128, 8 * RB], F32, name="urm", tag="urm")
        pid = nc.vector.partition_id()
        urt_r = urt.rearrange("p (m t) -> p m t", m=8)
        urm_r = urm.rearrange("p (m t) -> p m t", m=8)
        nc.vector.tensor_copy(urm_r[:, :, :], urt_r[:, :, ds(pid * RB, RB)])

        for i in range(RB):
            ps = pd.tile([2, T], F32, name="pdl", tag="pdl")
            nc.tensor.matmul(
                ps[:], bdp[:], ones[:], start=True, stop=False, skip_group_check=True
            )
            for m in range(8):
                rt = wk.tile([128, T], BF16, name="rt", tag="rt")
                src = ult[:, m * T : (m + 1) * T]
                bcol = urm[:, m * RB + i : m * RB + i + 1]
                if m < 6:
                    nc.vector.tensor_scalar(
                        rt[:], src, bcol, 0.0, ALU.add, ALU.max
                    )
                elif m == 6:
                    nc.gpsimd.tensor_scalar(
                        rt[:], src, bcol, 0.0, ALU.add, ALU.max
                    )
                else:
                    nc.scalar.activation(rt[:], src, AF.Relu, bias=bcol)
                nc.tensor.matmul(
                    ps[:],
                    wdp[:, m * 2 : (m + 1) * 2],
                    rt[:],
                    start=False,
                    stop=(m == 7),
                    skip_group_check=True,
                )
            # rows of ps: (Delta, -Delta); out1 = Delta - softplus(Delta),
            # out0 = -Delta - softplus(-Delta)  -> rows (out1, out0)
            ex = wk.tile([2, T], F32, name="ex", tag="ex")
            nc.scalar.activation(ex[:], ps[:], AF.Exp)
            ll = wk.tile([2, T], F32, name="ll", tag="ll")
            nc.scalar.activation(ll[:], ex[:], AF.Ln, bias=1.0)
            xo = wk.tile([2, T], F32, name="xo", tag="xo")
            nc.vector.tensor_tensor(xo[:], ps[:], ll[:], ALU.subtract)
            nc.sync.dma_start(OUT[:, i * T : (i + 1) * T], xo[:])

    nc.compile()
    return nc


def kernel(**inputs):
    return _kernel_impl(T=384, **inputs)


def _kernel_impl(T, v_r, v_l, Wih0, Whh0, bih0, bhh0, Wih1, Whh1, bih1, bhh1,
                 W1, b1, W2, b2, W3, b3, Wout, bout):
    RB = T // NCORES
    perm = _gate_perm()

    def bf(x):
        return np.ascontiguousarray(np.asarray(x, np.float32)).astype(BFNP)

    def f32(x):
        return np.ascontiguousarray(np.asarray(x, np.float32))

    def tiles_km(wt, nk, nm):
        outp = np.zeros((128, nk * nm * 128), np.float32)
        for k in range(nk):
            for m in range(nm):
                blk = wt[k * 128 : (k + 1) * 128, m * 128 : (m + 1) * 128]
                outp[: blk.shape[0], (k * nm + m) * 128 : (k * nm + m) * 128 + blk.shape[1]] = blk
        return outp

    v_r, v_l = np.asarray(v_r, np.float32), np.asarray(v_l, np.float32)
    Wih0, Whh0 = np.asarray(Wih0, np.float32), np.asarray(Whh0, np.float32)
    Wih1, Whh1 = np.asarray(Wih1, np.float32), np.asarray(Whh1, np.float32)
    b0 = np.asarray(bih0, np.float32) + np.asarray(bhh0, np.float32)
    b1r = np.asarray(bih1, np.float32) + np.asarray(bhh1, np.float32)
    W1, b1 = np.asarray(W1, np.float32), np.asarray(b1, np.float32)
    W2, b2 = np.asarray(W2, np.float32), np.asarray(b2, np.float32)
    W3, b3 = np.asarray(W3, np.float32), np.asarray(b3, np.float32)
    Wout, bout = np.asarray(Wout, np.float32), np.asarray(bout, np.float32)

    xt = np.concatenate([v_r.T, v_l.T], axis=1)
    wih0t = np.stack([Wih0[d][perm].T for d in (0, 1)])
    b0p = np.stack([b0[d][perm].reshape(8, 128).T for d in (0, 1)])
    whh0t = np.stack([tiles_km(Whh0[d][perm].T, 2, 8) for d in (0, 1)])
    wih1t = np.stack([tiles_km(Wih1[d][perm].T, 4, 8) for d in (0, 1)])
    b1rp = np.stack([b1r[d][perm].reshape(8, 128).T for d in (0, 1)])
    whh1t = np.stack([tiles_km(Whh1[d][perm].T, 2, 8) for d in (0, 1)])
    w1tt = tiles_km(W1.T, 4, 8)
    b1mp = b1.reshape(8, 128).T
    w2tt = tiles_km(W2.T, 8, 4)
    b2mp = b2.reshape(4, 128).T
    w3s = 0.5 * (W3[:, :H2] + W3[:, H2:]).T
    w3tt = tiles_km(w3s, 4, 8)
    b3p = b3.reshape(8, 128).T
    wd = Wout[1] - Wout[0]
    wdp = np.zeros((128, 16), np.float32)
    for m in range(8):
        wdp[:, m * 2] = wd[m * 128 : (m + 1) * 128]
        wdp[:, m * 2 + 1] = -wd[m * 128 : (m + 1) * 128]
    bd = float(bout[1] - bout[0])
    bdp = np.array([[bd, -bd]], np.float32)

    in_map = {
        "XT": bf(xt),
        "WIH0T": bf(wih0t),
        "WHH0T": bf(whh0t),
        "WIH1T": bf(wih1t),
        "WHH1T": bf(whh1t),
        "B0": f32(b0p),
        "B1R": f32(b1rp),
        "W1T": bf(w1tt),
        "B1M": f32(b1mp),
        "W2T": bf(w2tt),
        "B2M": f32(b2mp),
        "W3T": bf(w3tt),
        "B3": f32(b3p),
        "WDP": bf(wdp),
        "BDP": bf(bdp),
        "IDN": bf(np.eye(128, dtype=np.float32)),
    }

    if T not in _cache:
        _cache[T] = _build(T)
    nc = _cache[T]

    core_ids = list(range(NCORES))
    in_maps = [in_map for _ in core_ids]
    res = run_bass_kernel_spmd(nc, in_maps, core_ids)

    out = np.empty((T, T, 2), np.float32)
    for c in core_ids:
        o = res.results[c]["OUT"].reshape(2, RB, T)
        out[c * RB : (c + 1) * RB, :, 0] = o[1]
        out[c * RB : (c + 1) * RB, :, 1] = o[0]
    return out.reshape(T * T, 2)


# revision 10
# speedup vs baseline: 1.0172x; 1.0002x over previous
"""Trainium2 Bass kernel for BiLSTM pairwise model (nn_BiLSTM_45612552684167).

Strategy:
  - 2-layer bidirectional LSTM + MLP replicated on all 8 cores (the LSTM
    recurrence is latency-bound; replication avoids collectives entirely).
  - Pairwise [Nr, Nl] grid sharded along Nr via partition_id: each core
    computes its 48-row block of relu(u_r[i]+u_l[j]+b3) and the final
    projection, exploiting RRI=2: log_softmax reduces to softplus of
    Delta = logit1 - logit0 (single matvec against Wout[1]-Wout[0]).
  - bf16 on PE-facing tensors, f32 PSUM accumulation and elementwise.
Layouts keep every activation transposed ([feature-chunk(128), time/pair])
so ACT per-partition bias == feature bias and matmuls need no transposes.
"""

import sys
from contextlib import ExitStack

sys.path.insert(0, "/opt/trn_rl_repo")

import numpy as np
import ml_dtypes

import concourse.bass as bass
import concourse.mybir as mybir
import concourse.tile as tile
from concourse import bacc
from concourse.bass import ds
from concourse.bass_utils import run_bass_kernel_spmd

BFNP = ml_dtypes.bfloat16
F32 = mybir.dt.float32
BF16 = mybir.dt.bfloat16
AF = mybir.ActivationFunctionType
ALU = mybir.AluOpType

DIN = 22
H = 256
G = 1024  # 4*H
H1, H2, H3 = 1024, 512, 1024
NCORES = 8

_cache = {}


def _gate_perm():
    # torch gate order i,f,g,o -> device order f,g,i,o: (f,g) accumulate in PSUM
    # bank A, (i,o) in bank B so f/g nonlinearities overlap the i/o matmuls
    idx = np.arange(G).reshape(4, H)
    return np.concatenate([idx[1], idx[2], idx[0], idx[3]])


def _build(T):
    RB = T // NCORES
    nc = bacc.Bacc("TRN2", target_bir_lowering=False, debug=False, num_devices=NCORES)

    def inp(name, shape, dt):
        return nc.declare_dram_parameter(name, list(shape), dt, isOutput=False)

    XT = inp("XT", [DIN, 2 * T], BF16)
    WIH0T = inp("WIH0T", [2, DIN, G], BF16)
    WHH0T = inp("WHH0T", [2, 128, 2048], BF16)
    WIH1T = inp("WIH1T", [2, 128, 4096], BF16)
    WHH1T = inp("WHH1T", [2, 128, 2048], BF16)
    B0 = inp("B0", [2, 128, 8], F32)
    B1R = inp("B1R", [2, 128, 8], F32)
    W1T = inp("W1T", [128, 4096], BF16)  # tiles (k4, m8)
    B1M = inp("B1M", [128, 8], F32)
    W2T = inp("W2T", [128, 4096], BF16)  # tiles (k8, m4)
    B2M = inp("B2M", [128, 4], F32)
    W3T = inp("W3T", [128, 4096], BF16)  # tiles (k4, m8), pre-scaled 0.5
    B3 = inp("B3", [128, 8], F32)
    WDP = inp("WDP", [128, 16], BF16)  # per m-chunk: [wd, -wd]
    BDP = inp("BDP", [1, 2], BF16)  # [bd, -bd]
    IDN = inp("IDN", [128, 128], BF16)
    OUT = nc.declare_dram_parameter("OUT", [2, RB * T], F32, isOutput=True)

    with tile.TileContext(nc) as tc, ExitStack() as _es:
        sp = _es.enter_context(tc.tile_pool(name="static", bufs=1))
        wk = _es.enter_context(tc.tile_pool(name="work", bufs=4))
        pg = _es.enter_context(tc.tile_pool(name="psg", bufs=1, space="PSUM"))
        pb = _es.enter_context(tc.tile_pool(name="psb", bufs=2, space="PSUM"))
        pd = _es.enter_context(tc.tile_pool(name="psd", bufs=2, space="PSUM"))

        # ---- load all inputs to SBUF ----
        def load(name, dram_ap, shape, dt):
            t_ = sp.tile(shape, dt, tag=name)
            nc.sync.dma_start(t_[:], dram_ap)
            return t_

        xt = load("xt", XT[:, :], [DIN, 2 * T], BF16)
        wih0 = [load(f"wih0_{d}", WIH0T[d, :, :], [DIN, G], BF16) for d in (0, 1)]
        whh0 = [load(f"whh0_{d}", WHH0T[d, :, :], [128, 2048], BF16) for d in (0, 1)]
        wih1 = [load(f"wih1_{d}", WIH1T[d, :, :], [128, 4096], BF16) for d in (0, 1)]
        whh1 = [load(f"whh1_{d}", WHH1T[d, :, :], [128, 2048], BF16) for d in (0, 1)]
        b0 = [load(f"b0_{d}", B0[d, :, :], [128, 8], F32) for d in (0, 1)]
        b1r = [load(f"b1r_{d}", B1R[d, :, :], [128, 8], F32) for d in (0, 1)]
        w1t = load("w1t", W1T[:, :], [128, 4096], BF16)
        b1m = load("b1m", B1M[:, :], [128, 8], F32)
        w2t = load("w2t", W2T[:, :], [128, 4096], BF16)
        b2m = load("b2m", B2M[:, :], [128, 4], F32)
        w3t = load("w3t", W3T[:, :], [128, 4096], BF16)
        b3 = load("b3", B3[:, :], [128, 8], F32)
        wdp = load("wdp", WDP[:, :], [128, 16], BF16)
        bdp = load("bdp", BDP[:, :], [1, 2], BF16)
        idn = load("idn", IDN[:, :], [128, 128], BF16)

        ones = sp.tile([1, T], BF16, name="ones", tag="ones")
        nc.gpsimd.memset(ones[:], 1.0)

        pre_a = sp.tile([128, 32 * T], BF16, name="pre_a", tag="pre_a")
        pre_b = pre_a
        hist0 = [sp.tile([128, 4 * T], BF16, name=f"hist0_{d}", tag=f"hist0_{d}") for d in (0, 1)]
        hist1 = [sp.tile([128, 4 * T], BF16, name=f"hist1_{d}", tag=f"hist1_{d}") for d in (0, 1)]
        cst = [sp.tile([128, 4], F32, name=f"c_{d}", tag=f"c_{d}") for d in (0, 1)]

        def build_pre_a():
            # pre_a[:, t*32 + d*16 + m*2 + s] = (Wih0[d] @ x_s[t])[mchunk] + b0[d][mchunk]
            pre_r = pre_a.rearrange("p (t q) -> p t q", q=32)
            for d in (0, 1):
                for s in (0, 1):
                    for m in range(8):
                        ps = pb.tile([128, T], F32, name="big", tag="big")
                        nc.tensor.matmul(
                            ps[:],
                            wih0[d][:, m * 128 : (m + 1) * 128],
                            xt[:, s * T : (s + 1) * T],
                            start=True,
                            stop=True,
                        )
                        dst = pre_r[:, :, d * 16 + m * 2 + s]
                        if (d + s + m) % 2 == 0:
                            nc.scalar.activation(
                                dst, ps[:], AF.Identity, bias=b0[d][:, m : m + 1]
                            )
                        else:
                            nc.vector.tensor_scalar(
                                dst, ps[:], b0[d][:, m : m + 1], None, ALU.add
                            )

        def build_pre_b():
            # x1 = [h_fwd, h_bwd] per seq; pre_b from Wih1 + b1r
            pre_r = pre_b.rearrange("p (t q) -> p t q", q=32)
            h0r = [hist0[dd].rearrange("p (t q) -> p t q", q=4) for dd in (0, 1)]
            for d in (0, 1):
                for s in (0, 1):
                    for m in range(8):
                        ps = pb.tile([128, T], F32, name="big", tag="big")
                        for k in range(4):
                            rhs = h0r[k // 2][:, :, (k % 2) * 2 + s]
                            nc.tensor.matmul(
                                ps[:],
                                wih1[d][:, (k * 8 + m) * 128 : (k * 8 + m + 1) * 128],
                                rhs,
                                start=(k == 0),
                                stop=(k == 3),
                                skip_group_check=True,
                            )
                        dst = pre_r[:, :, d * 16 + m * 2 + s]
                        if (d + s + m) % 2 == 0:
                            nc.scalar.activation(
                                dst, ps[:], AF.Identity, bias=b1r[d][:, m : m + 1]
                            )
                        else:
                            nc.vector.tensor_scalar(
                                dst, ps[:], b1r[d][:, m : m + 1], None, ALU.add
                            )

        def lstm_phase(pre, whh, hist):
            for d in (0, 1):
                nc.gpsimd.memset(cst[d][:], 0.0)
            for t in range(T):
                for d in (0, 1):
                    tau = t if d == 0 else T - 1 - t
                    psa = pg.tile([128, 8], F32, name=f"ga{d}", tag=f"ga{d}")
                    psb = pg.tile([128, 8], F32, name=f"gb{d}", tag=f"gb{d}")
                    off = tau * 32 + d * 16
                    ptau = (tau - 1 if d == 0 else tau + 1) if t > 0 else 0
                    # group A: f,g gates (m0..3)
                    nc.tensor.matmul(
                        psa[:],
                        idn[:],
                        pre[:, off : off + 8],
                        start=True,
                        stop=(t == 0),
                        skip_group_check=True,
                    )
                    if t > 0:
                        for k in (0, 1):
                            rhs = hist[d][:, ptau * 4 + k * 2 : ptau * 4 + k * 2 + 2]
                            for m in range(4):
                                nc.tensor.matmul(
                                    psa[:, m * 2 : m * 2 + 2],
                                    whh[d][:, (k * 8 + m) * 128 : (k * 8 + m + 1) * 128],
                                    rhs,
                                    start=False,
                                    stop=(k == 1 and m == 3),
                                    skip_group_check=True,
                                )
                    # group B: i,o gates (m4..7)
                    nc.tensor.matmul(
                        psb[:],
                        idn[:],
                        pre[:, off + 8 : off + 16],
                        start=True,
                        stop=(t == 0),
                        skip_group_check=True,
                    )
                    if t > 0:
                        for k in (0, 1):
                            rhs = hist[d][:, ptau * 4 + k * 2 : ptau * 4 + k * 2 + 2]
                            for m in range(4, 8):
                                nc.tensor.matmul(
                                    psb[:, (m - 4) * 2 : (m - 4) * 2 + 2],
                                    whh[d][:, (k * 8 + m) * 128 : (k * 8 + m + 1) * 128],
                                    rhs,
                                    start=False,
                                    stop=(k == 1 and m == 7),
                                    skip_group_check=True,
                                )
                    # gsb layout: f[0:4] g[4:8] i[8:12] o[12:16]
                    gsb = wk.tile([128, 16], F32, name=f"gs{d}", tag=f"gs{d}")
                    nc.scalar.activation(gsb[:, 0:4], psa[:, 0:4], AF.Sigmoid)
                    nc.scalar.activation(gsb[:, 4:8], psa[:, 4:8], AF.Tanh)
                    nc.vector.tensor_tensor(
                        cst[d][:], gsb[:, 0:4], cst[d][:], ALU.mult
                    )
                    nc.scalar.activation(gsb[:, 8:16], psb[:], AF.Sigmoid)
                    tmp = wk.tile([128, 4], F32, name=f"tmp{d}", tag=f"tmp{d}")
                    nc.vector.tensor_tensor(
                        tmp[:], gsb[:, 8:12], gsb[:, 4:8], ALU.mult
                    )
                    nc.vector.tensor_tensor(cst[d][:], cst[d][:], tmp[:], ALU.add)
                    tch = wk.tile([128, 4], F32, name=f"tc{d}", tag=f"tc{d}")
                    nc.scalar.activation(tch[:], cst[d][:], AF.Tanh)
                    nc.vector.tensor_tensor(
                        hist[d][:, tau * 4 : tau * 4 + 4],
                        gsb[:, 12:16],
                        tch[:],
                        ALU.mult,
                    )

        build_pre_a()
        lstm_phase(pre_a, whh0, hist0)
        build_pre_b()
        lstm_phase(pre_b, whh1, hist1)

        # ---- MLP (transposed activations) ----
        h1t = [sp.tile([128, 8 * T], BF16, name=f"h1t_{s}", tag=f"h1t_{s}") for s in (0, 1)]
        h2t = [sp.tile([128, 4 * T], BF16, name=f"h2t_{s}", tag=f"h2t_{s}") for s in (0, 1)]
        urt = sp.tile([128, 8 * T], BF16, name="urt", tag="urt")
        ult = sp.tile([128, 8 * T], BF16, name="ult", tag="ult")
        h1r = [hist1[dd].rearrange("p (t q) -> p t q", q=4) for dd in (0, 1)]

        for s in (0, 1):
            for m in range(8):
                ps = pb.tile([128, T], F32, name="big", tag="big")
                for k in range(4):
                    rhs = h1r[k // 2][:, :, (k % 2) * 2 + s]
                    nc.tensor.matmul(
                        ps[:],
                        w1t[:, (k * 8 + m) * 128 : (k * 8 + m + 1) * 128],
                        rhs,
                        start=(k == 0),
                        stop=(k == 3),
                        skip_group_check=True,
                    )
                nc.scalar.activation(
                    h1t[s][:, m * T : (m + 1) * T],
                    ps[:],
                    AF.Relu,
                    bias=b1m[:, m : m + 1],
                )
            for m in range(4):
                ps = pb.tile([128, T], F32, name="big", tag="big")
                for k in range(8):
                    nc.tensor.matmul(
                        ps[:],
                        w2t[:, (k * 4 + m) * 128 : (k * 4 + m + 1) * 128],
                        h1t[s][:, k * T : (k + 1) * T],
                        start=(k == 0),
                        stop=(k == 7),
                        skip_group_check=True,
                    )
                nc.scalar.activation(
                    h2t[s][:, m * T : (m + 1) * T],
                    ps[:],
                    AF.Relu,
                    bias=b2m[:, m : m + 1],
                )
            dst_u = urt if s == 0 else ult
            for m in range(8):
                ps = pb.tile([128, T], F32, name="big", tag="big")
                for k in range(4):
                    nc.tensor.matmul(
                        ps[:],
                        w3t[:, (k * 8 + m) * 128 : (k * 8 + m + 1) * 128],
                        h2t[s][:, k * T : (k + 1) * T],
                        start=(k == 0),
                        stop=(k == 3),
                        skip_group_check=True,
                    )
                if s == 1:
                    nc.scalar.activation(
                        dst_u[:, m * T : (m + 1) * T],
                        ps[:],
                        AF.Identity,
                        bias=b3[:, m : m + 1],
                    )
                else:
                    nc.scalar.activation(
                        dst_u[:, m * T : (m + 1) * T], ps[:], AF.Identity, bias=0.0
                    )

        # ---- pairwise block (rows pid*RB .. pid*RB+RB-1) ----
        urm = sp.tile([128, 8 * RB], F32, name="urm", tag="urm")
        pid = nc.vector.partition_id()
        urt_r = urt.rearrange("p (m t) -> p m t", m=8)
        urm_r = urm.rearrange("p (m t) -> p m t", m=8)
        nc.vector.tensor_copy(urm_r[:, :, :], urt_r[:, :, ds(pid * RB, RB)])

        for i in range(RB):
            ps = pd.tile([2, T], F32, name="pdl", tag="pdl")
            nc.tensor.matmul(
                ps[:], bdp[:], ones[:], start=True, stop=False, skip_group_check=True
            )
            for m in range(8):
                rt = wk.tile([128, T], BF16, name="rt", tag="rt")
                src = ult[:, m * T : (m + 1) * T]
                bcol = urm[:, m * RB + i : m * RB + i + 1]
                if m < 6:
                    nc.vector.tensor_scalar(
                        rt[:], src, bcol, 0.0, ALU.add, ALU.max
                    )
                elif m == 6:
                    nc.gpsimd.tensor_scalar(
                        rt[:], src, bcol, 0.0, ALU.add, ALU.max
                    )
                else:
                    nc.scalar.activation(rt[:], src, AF.Relu, bias=bcol)
                nc.tensor.matmul(
                    ps[:],
                    wdp[:, m * 2 : (m + 1) * 2],
                    rt[:],
                    start=False,
                    stop=(m == 7),
                    skip_group_check=True,
                )
            # rows of ps: (Delta, -Delta); out1 = Delta - softplus(Delta),
            # out0 = -Delta - softplus(-Delta)  -> rows (out1, out0)
            ex = wk.tile([2, T], F32, name="ex", tag="ex")
            nc.scalar.activation(ex[:], ps[:], AF.Exp)
            ll = wk.tile([2, T], F32, name="ll", tag="ll")
            nc.scalar.activation(ll[:], ex[:], AF.Ln, bias=1.0)
            xo = wk.tile([2, T], F32, name="xo", tag="xo")
            nc.vector.tensor_tensor(xo[:], ps[:], ll[:], ALU.subtract)
            nc.sync.dma_start(OUT[:, i * T : (i + 1) * T], xo[:])

    nc.compile()
    return nc


def kernel(**inputs):
    return _kernel_impl(T=384, **inputs)


def _kernel_impl(T, v_r, v_l, Wih0, Whh0, bih0, bhh0, Wih1, Whh1, bih1, bhh1,
                 W1, b1, W2, b2, W3, b3, Wout, bout):
    RB = T // NCORES
    perm = _gate_perm()

    def bf(x):
        return np.ascontiguousarray(np.asarray(x, np.float32)).astype(BFNP)

    def f32(x):
        return np.ascontiguousarray(np.asarray(x, np.float32))

    def tiles_km(wt, nk, nm):
        outp = np.zeros((128, nk * nm * 128), np.float32)
        for k in range(nk):
            for m in range(nm):
                blk = wt[k * 128 : (k + 1) * 128, m * 128 : (m + 1) * 128]
                outp[: blk.shape[0], (k * nm + m) * 128 : (k * nm + m) * 128 + blk.shape[1]] = blk
        return outp

    v_r, v_l = np.asarray(v_r, np.float32), np.asarray(v_l, np.float32)
    Wih0, Whh0 = np.asarray(Wih0, np.float32), np.asarray(Whh0, np.float32)
    Wih1, Whh1 = np.asarray(Wih1, np.float32), np.asarray(Whh1, np.float32)
    b0 = np.asarray(bih0, np.float32) + np.asarray(bhh0, np.float32)
    b1r = np.asarray(bih1, np.float32) + np.asarray(bhh1, np.float32)
    W1, b1 = np.asarray(W1, np.float32), np.asarray(b1, np.float32)
    W2, b2 = np.asarray(W2, np.float32), np.asarray(b2, np.float32)
    W3, b3 = np.asarray(W3, np.float32), np.asarray(b3, np.float32)
    Wout, bout = np.asarray(Wout, np.float32), np.asarray(bout, np.float32)

    xt = np.concatenate([v_r.T, v_l.T], axis=1)
    wih0t = np.stack([Wih0[d][perm].T for d in (0, 1)])
    b0p = np.stack([b0[d][perm].reshape(8, 128).T for d in (0, 1)])
    whh0t = np.stack([tiles_km(Whh0[d][perm].T, 2, 8) for d in (0, 1)])
    wih1t = np.stack([tiles_km(Wih1[d][perm].T, 4, 8) for d in (0, 1)])
    b1rp = np.stack([b1r[d][perm].reshape(8, 128).T for d in (0, 1)])
    whh1t = np.stack([tiles_km(Whh1[d][perm].T, 2, 8) for d in (0, 1)])
    w1tt = tiles_km(W1.T, 4, 8)
    b1mp = b1.reshape(8, 128).T
    w2tt = tiles_km(W2.T, 8, 4)
    b2mp = b2.reshape(4, 128).T
    w3s = 0.5 * (W3[:, :H2] + W3[:, H2:]).T
    w3tt = tiles_km(w3s, 4, 8)
    b3p = b3.reshape(8, 128).T
    wd = Wout[1] - Wout[0]
    wdp = np.zeros((128, 16), np.float32)
    for m in range(8):
        wdp[:, m * 2] = wd[m * 128 : (m + 1) * 128]
        wdp[:, m * 2 + 1] = -wd[m * 128 : (m + 1) * 128]
    bd = float(bout[1] - bout[0])
    bdp = np.array([[bd, -bd]], np.float32)

    in_map = {
        "XT": bf(xt),
        "WIH0T": bf(wih0t),
        "WHH0T": bf(whh0t),
        "WIH1T": bf(wih1t),
        "WHH1T": bf(whh1t),
        "B0": f32(b0p),
        "B1R": f32(b1rp),
        "W1T": bf(w1tt),
        "B1M": f32(b1mp),
        "W2T": bf(w2tt),
        "B2M": f32(b2mp),
        "W3T": bf(w3tt),
        "B3": f32(b3p),
        "WDP": bf(wdp),
        "BDP": bf(bdp),
        "IDN": bf(np.eye(128, dtype=np.float32)),
    }

    if T not in _cache:
        _cache[T] = _build(T)
    nc = _cache[T]

    core_ids = list(range(NCORES))
    in_maps = [in_map for _ in core_ids]
    res = run_bass_kernel_spmd(nc, in_maps, core_ids)

    out = np.empty((T, T, 2), np.float32)
    for c in core_ids:
        o = res.results[c]["OUT"].reshape(2, RB, T)
        out[c * RB : (c + 1) * RB, :, 0] = o[1]
        out[c * RB : (c + 1) * RB, :, 1] = o[0]
    return out.reshape(T * T, 2)
